# revision 20
# baseline (speedup 1.0000x reference)
"""Trainium2 Bass kernel for nn_AttentionBlock (b,h,w,c = 32,64,64,256).

out = x + (softmax_w(QK^T * s) @ V) @ Wo + bo   with Q/K/V = x@W* + b*
per-row attention over the w axis, batch-parallel over 8 NeuronCores.

Algebra (validated against the jax reference):
  scores*s = x A x^T + ones_i (x wv)^T   where A = (Wq Wk^T) s, wv = (Wk bq) s
  (the i-side bias term and the constant cancel inside softmax)
  out = attn @ x @ U + (bv@Wo + bo) + x  with U = Wv Wo  (V and output
  projections fused on the output side).
The attention matmul's moving operand is [xu | 1], so the softmax
row-sum falls out as output column 256 of the same matmul.
x is loaded as bf16 via casting SWDGE DMAs (gpsimd queue, so output
stores on the sync queue never block the prefetch); all PE operands are
bf16, PSUM accumulates fp32, the bf16 residual costs ~2e-3 relative
error against the 2e-2 budget.  Outputs are fp32.
"""

import os
import sys

for _p in ("/opt/trn_rl_repo", os.path.expanduser("~/.axon_site/_ro/trn_rl_repo")):
    if os.path.isdir(_p) and _p not in sys.path:
        sys.path.append(_p)

import numpy as np

import concourse.bass as bass
import concourse.mybir as mybir
import concourse.tile as tile
from concourse import bacc
from concourse.masks import make_identity

N_CORES = 8
B, H, W, C = 32, 64, 64, 256
BPC = B // N_CORES            # batch images per core
RPC = BPC * H * W             # rows per core = 16384
SCALE = 1.0 / (C * np.sqrt(0.5) * np.sqrt(C))   # folded softmax scale

F32 = mybir.dt.float32
BF16 = mybir.dt.bfloat16

CHUNK = 2048                  # rows per chunk (32 attention pairs)
N_RT = CHUNK // 128           # row-tiles per chunk
XU_W = 264                    # xu row stride (257 used; 264 for alignment)


def _build_body(nc, tc, x_d, w_d, b_d, out_d, n_chunks, ctx, use_bias):
    """Emit the kernel body. w_d/b_d: dicts of weight/bias dram handles."""

    def pool(name, bufs, space="SBUF"):
        kw = {} if space == "SBUF" else {"space": bass.MemorySpace.PSUM}
        return ctx.enter_context(tc.tile_pool(name=name, bufs=bufs, **kw))

    const = pool("const", 1)
    wtmp = pool("wtmp", 2)
    # PSUM: every slot is one full bank; 1+2+2+2+1 = 8 banks exactly
    ptx = pool("ptx", 1, "PSUM")      # [128,4,2,128] bf16: 4 rt of transposes
    pxu = pool("pxu", 1, "PSUM")      # [128,2,256] f32: xu pairs
    psc = pool("psc", 2, "PSUM")      # [128,4,128] f32: score super-tiles
    ppo = pool("ppo", 2, "PSUM")      # [128,257] f32: attention out + rowsum
    pgv = pool("pgv", 2, "PSUM")      # [128,512] f32: G blocks (+preamble)

    # ---------------- preamble: constants & weight prep ----------------
    ident_b = const.tile([128, 128], BF16, tag="identb")
    make_identity(nc, ident_b)
    ones_b = const.tile([1, 128], BF16, tag="ones")
    nc.vector.memset(ones_b, 1.0)

    # transposed Wq/Wk/Wv: WT[:, dc, c] = W[c, 128*dc + d]
    WqT = const.tile([128, 2, 256], BF16, tag="wqt")
    WkT = const.tile([128, 2, 256], BF16, tag="wkt")
    WvT = const.tile([128, 2, 256], BF16, tag="wvt")
    Wo_b = const.tile([128, 2, 256], BF16, tag="wob")
    for wname, wt in (("Wq", WqT), ("Wk", WkT), ("Wv", WvT)):
        for cc in range(2):
            wrow = wtmp.tile([128, 256], F32, tag="wrow")
            nc.sync.dma_start(out=wrow, in_=w_d[wname][cc * 128:(cc + 1) * 128, :])
            wrow_b = wtmp.tile([128, 256], BF16, tag="wrowb")
            nc.vector.tensor_copy(wrow_b, wrow)
            tp = ptx.tile([128, 4, 2, 128], BF16, tag="tx")
            for dc in range(2):
                nc.tensor.transpose(tp[:, 0, dc, :],
                                    wrow_b[:, dc * 128:(dc + 1) * 128], ident_b)
                nc.any.tensor_copy(wt[:, dc, cc * 128:(cc + 1) * 128],
                                   tp[:, 0, dc, :])
    for wname, wt in (("Wo", Wo_b),):
        for cc in range(2):
            wrow = wtmp.tile([128, 256], F32, tag="wrow")
            nc.sync.dma_start(out=wrow, in_=w_d[wname][cc * 128:(cc + 1) * 128, :])
            nc.any.tensor_copy(wt[:, cc, :], wrow)

    # A[c, a] = s * sum_d Wq[c, d] Wk[a, d]
    A_b = const.tile([128, 2, 256], BF16, tag="ab")
    for cc in range(2):
        pa = pgv.tile([128, 512], F32, tag="g")
        for dc in range(2):
            nc.tensor.matmul(pa[:, 0:256], WqT[:, dc, cc * 128:(cc + 1) * 128],
                             WkT[:, dc, :], start=(dc == 0), stop=(dc == 1))
        nc.any.tensor_scalar_mul(A_b[:, cc, :], pa[:, 0:256], float(SCALE))

    # U[c, e] = sum_d Wv[c, d] Wo[d, e]  (fused V+output projection)
    U_b = const.tile([128, 2, 256], BF16, tag="ub")
    for cc in range(2):
        pu = pgv.tile([128, 512], F32, tag="g")
        for dc in range(2):
            nc.tensor.matmul(pu[:, 0:256], WvT[:, dc, cc * 128:(cc + 1) * 128],
                             Wo_b[:, dc, :], start=(dc == 0), stop=(dc == 1))
        nc.any.tensor_copy(U_b[:, cc, :], pu[:, 0:256])

    wv_b = bo2_bc = None
    if use_bias:
        bq_b = const.tile([128, 2], BF16, tag="bqb")
        bv_b = const.tile([128, 2], BF16, tag="bvb")
        for bname, bt in (("bq", bq_b), ("bv", bv_b)):
            bf = wtmp.tile([128, 2], F32, tag="bcol")
            for cc in range(2):
                nc.sync.dma_start(out=bf[:, cc:cc + 1],
                                  in_=b_d[bname][cc * 128:(cc + 1) * 128].unsqueeze(1))
            nc.any.tensor_copy(bt, bf)

        # wv[c] = s * sum_d Wk[c, d] bq[d]
        wv_b = const.tile([128, 2], BF16, tag="wvvec")
        for cc in range(2):
            pwv = psc.tile([128, 4, 128], F32, tag="sc")
            for dc in range(2):
                nc.tensor.matmul(pwv[:, 0, 0:1], WkT[:, dc, cc * 128:(cc + 1) * 128],
                                 bq_b[:, dc:dc + 1], start=(dc == 0), stop=(dc == 1))
            nc.any.tensor_scalar_mul(wv_b[:, cc:cc + 1], pwv[:, 0, 0:1], float(SCALE))

        # bo2[e] = bv @ Wo + bo  (fused output bias)
        bo_f = wtmp.tile([1, 256], F32, tag="borow")
        nc.sync.dma_start(out=bo_f, in_=b_d["bo"][:].unsqueeze(0))
        pb = pgv.tile([128, 512], F32, tag="g")
        for cc in range(2):
            nc.tensor.matmul(pb[0:1, 0:256], bv_b[:, cc:cc + 1], Wo_b[:, cc, :],
                             start=(cc == 0), stop=(cc == 1))
        bo2_f = wtmp.tile([1, 256], F32, tag="bo2row")
        nc.vector.tensor_add(bo2_f, pb[0:1, 0:256], bo_f)
        bo2_dram = nc.dram_tensor("bo2_bounce", [256], F32)
        nc.sync.dma_start(out=bo2_dram[:].unsqueeze(0), in_=bo2_f)
        bo2_bc = const.tile([128, 256], F32, tag="bo2bc")
        bo2_src = bass.AP(tensor=bo2_dram, offset=0,
                          ap=[[0, 128], [1, 256]])
        nc.sync.dma_start(out=bo2_bc, in_=bo2_src)

    # ---------------- main loop ----------------
    # Three phases per chunk: A = load/transpose/G/xU, B = scores+exp,
    # C = attention ladder (attn matmul, 1/rowsum, residual, store).
    # C(ch) is latency-bound (each step waits on the previous engine), so
    # it is interleaved with A(ch+1) at 4-row-tile granularity: the PE
    # always has ready A-work between dependent C-matmuls.
    xbpool = pool("xb", 3)
    xtpool = pool("xt", 2)
    gtpool = pool("gt", 2)
    xupool = pool("xu", 2)
    expool = pool("expt", 9)
    rspool = pool("rs", 6)
    outpool = pool("outs", 3)

    NG = N_RT // 4            # 4-row-tile groups per chunk

    state = {}

    def emit_load(ch):
        r0 = ch * CHUNK
        xb = xbpool.tile([128, N_RT, 256], BF16, tag="xball")
        for g in range(N_RT // 4):
            rr = r0 + g * 512
            nc.gpsimd.dma_start(
                out=xb[:, g * 4:(g + 1) * 4, :],
                in_=x_d[rr:rr + 512, :].rearrange("(t p) c -> p t c", p=128))
        xT = xtpool.tile([128, 2, CHUNK], BF16, tag="xt")
        GT = gtpool.tile([128, 2, CHUNK], BF16, tag="gt")
        xu = xupool.tile([128, N_RT, XU_W], BF16, tag="xub")
        st = {"xb": xb, "r0": r0, "xT": xT, "GT": GT, "xu": xu,
              "expT": [], "xv": None}
        nc.gpsimd.memset(st["xu"][:, :, 256:257], 1.0)
        if use_bias:
            xv_b = xupool.tile([1, CHUNK], BF16, tag="xvb")
            st["xv"] = xv_b
        state[ch] = st
        return st

    def emit_A_unit(ch, g):
        """transposes + G block + xU matmuls for 4 row-tiles."""
        st = state[ch]
        xb, xT, GT, xu = st["xb"], st["xT"], st["GT"], st["xu"]
        rt0 = g * 4
        tpx = ptx.tile([128, 4, 2, 128], BF16, tag="tx")
        for r in range(4):
            for cc in range(2):
                nc.tensor.transpose(tpx[:, r, cc, :],
                                    xb[:, rt0 + r, cc * 128:(cc + 1) * 128],
                                    ident_b)
        dst = xT[:, :, rt0 * 128:(rt0 + 4) * 128].rearrange(
            "p c (r u) -> p r c u", r=4).bitcast(F32)
        if g % 2 == 0:
            nc.scalar.copy(dst, tpx.bitcast(F32))
        else:
            nc.vector.tensor_copy(dst, tpx.bitcast(F32))

        cs = g * 512
        for ac in range(2):
            pgt = pgv.tile([128, 512], F32, tag="g")
            for cc in range(2):
                nc.tensor.matmul(pgt, A_b[:, cc, ac * 128:(ac + 1) * 128],
                                 xT[:, cc, cs:cs + 512],
                                 start=(cc == 0), stop=(cc == 1))
            if ac == 0:
                nc.vector.tensor_copy(GT[:, ac, cs:cs + 512], pgt)
            else:
                nc.scalar.copy(GT[:, ac, cs:cs + 512], pgt)
        if use_bias:
            pxv = pgv.tile([128, 512], F32, tag="g")
            for cc in range(2):
                nc.tensor.matmul(pxv[0:1, :], wv_b[:, cc:cc + 1],
                                 xT[:, cc, cs:cs + 512],
                                 start=(cc == 0), stop=(cc == 1))
            nc.any.tensor_copy(st["xv"][0:1, cs:cs + 512], pxv[0:1, :])

        for r in range(4):
            rt = rt0 + r
            if rt % 2 == 0:
                pxt = pxu.tile([128, 2, 256], F32, tag="xup")
                st["pxt"] = pxt
            ir = rt * 128
            for cc in range(2):
                nc.tensor.matmul(st["pxt"][:, rt % 2, :], xT[:, cc, ir:ir + 128],
                                 U_b[:, cc, :], start=(cc == 0), stop=(cc == 1))
            if rt % 2 == 1:
                dst = xu[:, rt - 1:rt + 1, 0:256]
                if (rt // 2) % 2 == 0:
                    nc.vector.tensor_copy(dst, st["pxt"])
                else:
                    nc.scalar.copy(dst, st["pxt"])

    def emit_B_unit(ch, sg):
        """scores + exp for one 4-row-tile super-group."""
        st = state[ch]
        xT, GT = st["xT"], st["GT"]
        scT4 = psc.tile([128, 4, 128], F32, tag="sc")
        nmm = 12 if use_bias else 8
        mi = 0
        for r in range(4):
            ir = (sg * 4 + r) * 128
            for ac in range(2):
                nc.tensor.matmul(scT4[:, r, :], xT[:, ac, ir:ir + 128],
                                 GT[:, ac, ir:ir + 128],
                                 start=(mi == 0), stop=(mi == nmm - 1))
                mi += 1
            if use_bias:
                nc.tensor.matmul(scT4[:, r, :], st["xv"][0:1, ir:ir + 128],
                                 ones_b, start=False, stop=(mi == nmm - 1))
                mi += 1
        expT4 = expool.tile([128, 4, 128], BF16, tag="expt")
        nc.gpsimd.memset(expT4, 0.0)
        for il in range(2):
            dg = slice(il * 64, (il + 1) * 64)
            nc.scalar.activation(expT4[dg, :, dg], scT4[dg, :, dg],
                                 mybir.ActivationFunctionType.Exp)
        st["expT"].append(expT4)

    def emit_C_unit(ch, sg):
        """attention ladder for one 4-row-tile super-group."""
        st = state[ch]
        for r in range(4):
            rt = sg * 4 + r
            pO = ppo.tile([128, 257], F32, tag="po")
            nc.tensor.matmul(pO, st["expT"][sg][:, r, :],
                             st["xu"][:, rt, 0:257], start=True, stop=True)
            rrs_col = rspool.tile([128, 1], F32, tag="rrs")
            nc.vector.reciprocal(rrs_col, pO[:, 256:257])
            if rt % 2 == 0:
                o_sb2 = outpool.tile([128, 2, 256], F32, tag="osb")
                st["osb"] = o_sb2
            nc.vector.scalar_tensor_tensor(
                st["osb"][:, rt % 2, :], pO[:, 0:256], rrs_col,
                st["xb"][:, rt, :],
                op0=mybir.AluOpType.mult, op1=mybir.AluOpType.add)
            if use_bias:
                nc.gpsimd.tensor_add(st["osb"][:, rt % 2, :],
                                     st["osb"][:, rt % 2, :], bo2_bc)
            if rt % 2 == 1:
                rr = st["r0"] + (rt - 1) * 128
                nc.sync.dma_start(
                    out=out_d[rr:rr + 256, :].rearrange(
                        "(t p) c -> p t c", p=128),
                    in_=st["osb"])

    # prologue: A(0), B(0)
    emit_load(0)
    for g in range(NG):
        emit_A_unit(0, g)
    for sg in range(NG):
        emit_B_unit(0, sg)

    for ch in range(n_chunks):
        nxt = ch + 1 if ch + 1 < n_chunks else None
        if nxt is not None:
            emit_load(nxt)
        for g in range(NG):
            emit_C_unit(ch, g)
            if nxt is not None:
                emit_A_unit(nxt, g)
        if nxt is not None:
            for sg in range(NG):
                emit_B_unit(nxt, sg)
        del state[ch]


def build(n_chunks=RPC // CHUNK, use_bias=True):
    nc = bacc.Bacc("TRN2", target_bir_lowering=False, debug=False)
    rows = n_chunks * CHUNK
    x_d = nc.declare_dram_parameter("x", [rows, C], F32, isOutput=False)
    w_d = {n: nc.declare_dram_parameter(n, [C, C], F32, isOutput=False)
           for n in ("Wq", "Wk", "Wv", "Wo")}
    b_d = {n: nc.declare_dram_parameter(n, [C], F32, isOutput=False)
           for n in ("bq", "bk", "bv", "bo")}
    out_d = nc.declare_dram_parameter("out", [rows, C], F32, isOutput=True)
    from contextlib import ExitStack
    with tile.TileContext(nc) as tc, ExitStack() as ctx:
        _build_body(nc, tc, x_d, w_d, b_d, out_d, n_chunks, ctx, use_bias)
    nc.compile()
    return nc


_NC = {}
TRACE = False
LAST_RESULT = None


def kernel(x, Wq, bq, Wk, bk, Wv, bv, Wo, bo):
    global LAST_RESULT
    use_bias = any(np.any(np.asarray(b)) for b in (bq, bk, bv, bo))
    if use_bias not in _NC:
        _NC[use_bias] = build(use_bias=use_bias)
    nc_k = _NC[use_bias]
    from concourse.bass_utils import run_bass_kernel_spmd

    x = np.ascontiguousarray(np.asarray(x, dtype=np.float32))
    shared = {
        "Wq": np.ascontiguousarray(Wq, dtype=np.float32),
        "Wk": np.ascontiguousarray(Wk, dtype=np.float32),
        "Wv": np.ascontiguousarray(Wv, dtype=np.float32),
        "Wo": np.ascontiguousarray(Wo, dtype=np.float32),
        "bq": np.ascontiguousarray(bq, dtype=np.float32),
        "bk": np.ascontiguousarray(bk, dtype=np.float32),
        "bv": np.ascontiguousarray(bv, dtype=np.float32),
        "bo": np.ascontiguousarray(bo, dtype=np.float32),
    }
    in_maps = []
    for i in range(N_CORES):
        xs = np.ascontiguousarray(
            x[i * BPC:(i + 1) * BPC].reshape(RPC, C))
        in_maps.append({"x": xs, **shared})
    res = run_bass_kernel_spmd(nc_k, in_maps, core_ids=list(range(N_CORES)),
                               trace=TRACE)
    LAST_RESULT = res
    out = np.concatenate(
        [res.results[i]["out"].reshape(BPC, H, W, C) for i in range(N_CORES)],
        axis=0)
    return out


# revision 21
# speedup vs baseline: 1.1362x; 1.1362x over previous
"""Trainium2 Bass kernel for nn_AttentionBlock (b,h,w,c = 32,64,64,256).

out = x + (softmax_w(QK^T * s) @ V) @ Wo + bo   with Q/K/V = x@W* + b*
per-row attention over the w axis, batch-parallel over 8 NeuronCores.

Algebra (validated against the jax reference):
  scores*s = x A x^T + ones_i (x wv)^T   where A = (Wq Wk^T) s, wv = (Wk bq) s
  (the i-side bias term and the constant cancel inside softmax)
  out = attn @ x @ U + (bv@Wo + bo) + x  with U = Wv Wo  (V and output
  projections fused on the output side).
The attention matmul's moving operand is [xu | 1], so the softmax
row-sum falls out as output column 256 of the same matmul.
x is loaded as bf16 via casting SWDGE DMAs (gpsimd queue, so output
stores on the sync queue never block the prefetch); all PE operands are
bf16, PSUM accumulates fp32, the bf16 residual costs ~2e-3 relative
error against the 2e-2 budget.  Outputs are fp32.
"""

import os
import sys

for _p in ("/opt/trn_rl_repo", os.path.expanduser("~/.axon_site/_ro/trn_rl_repo")):
    if os.path.isdir(_p) and _p not in sys.path:
        sys.path.append(_p)

import numpy as np

import concourse.bass as bass
import concourse.mybir as mybir
import concourse.tile as tile
from concourse import bacc
from concourse.masks import make_identity

N_CORES = 8
B, H, W, C = 32, 64, 64, 256
BPC = B // N_CORES            # batch images per core
RPC = BPC * H * W             # rows per core = 16384
SCALE = 1.0 / (C * np.sqrt(0.5) * np.sqrt(C))   # folded softmax scale

F32 = mybir.dt.float32
BF16 = mybir.dt.bfloat16

CHUNK = 2048                  # rows per chunk (32 attention pairs)
N_RT = CHUNK // 128           # row-tiles per chunk
XU_W = 264                    # xu row stride (257 used; 264 for alignment)


def _build_body(nc, tc, x_d, w_d, b_d, out_d, n_chunks, ctx, use_bias):
    """Emit the kernel body. w_d/b_d: dicts of weight/bias dram handles."""

    def pool(name, bufs, space="SBUF"):
        kw = {} if space == "SBUF" else {"space": bass.MemorySpace.PSUM}
        return ctx.enter_context(tc.tile_pool(name=name, bufs=bufs, **kw))

    const = pool("const", 1)
    wtmp = pool("wtmp", 2)
    # PSUM: every slot is one full bank; 1+2+2+2+1 = 8 banks exactly
    ptx = pool("ptx", 1, "PSUM")      # [128,4,2,128] bf16: 4 rt of transposes
    pxu = pool("pxu", 1, "PSUM")      # [128,2,256] f32: xu pairs
    psc = pool("psc", 2, "PSUM")      # [128,4,128] f32: score super-tiles
    ppo = pool("ppo", 3, "PSUM")      # [128,257] f32: attention out + rowsum
    pgv = pool("pgv", 1, "PSUM")      # [128,512] f32: G blocks (+preamble)

    # ---------------- preamble: constants & weight prep ----------------
    ident_b = const.tile([128, 128], BF16, tag="identb")
    make_identity(nc, ident_b)
    ones_b = const.tile([1, 128], BF16, tag="ones")
    nc.vector.memset(ones_b, 1.0)

    # transposed Wq/Wk/Wv: WT[:, dc, c] = W[c, 128*dc + d]
    WqT = const.tile([128, 2, 256], BF16, tag="wqt")
    WkT = const.tile([128, 2, 256], BF16, tag="wkt")
    WvT = const.tile([128, 2, 256], BF16, tag="wvt")
    Wo_b = const.tile([128, 2, 256], BF16, tag="wob")
    for wname, wt in (("Wq", WqT), ("Wk", WkT), ("Wv", WvT)):
        for cc in range(2):
            wrow = wtmp.tile([128, 256], F32, tag="wrow")
            nc.sync.dma_start(out=wrow, in_=w_d[wname][cc * 128:(cc + 1) * 128, :])
            wrow_b = wtmp.tile([128, 256], BF16, tag="wrowb")
            nc.vector.tensor_copy(wrow_b, wrow)
            tp = ptx.tile([128, 4, 2, 128], BF16, tag="tx")
            for dc in range(2):
                nc.tensor.transpose(tp[:, 0, dc, :],
                                    wrow_b[:, dc * 128:(dc + 1) * 128], ident_b)
                nc.any.tensor_copy(wt[:, dc, cc * 128:(cc + 1) * 128],
                                   tp[:, 0, dc, :])
    for wname, wt in (("Wo", Wo_b),):
        for cc in range(2):
            wrow = wtmp.tile([128, 256], F32, tag="wrow")
            nc.sync.dma_start(out=wrow, in_=w_d[wname][cc * 128:(cc + 1) * 128, :])
            nc.any.tensor_copy(wt[:, cc, :], wrow)

    # A[c, a] = s * sum_d Wq[c, d] Wk[a, d]
    A_b = const.tile([128, 2, 256], BF16, tag="ab")
    for cc in range(2):
        pa = pgv.tile([128, 512], F32, tag="g")
        for dc in range(2):
            nc.tensor.matmul(pa[:, 0:256], WqT[:, dc, cc * 128:(cc + 1) * 128],
                             WkT[:, dc, :], start=(dc == 0), stop=(dc == 1))
        nc.any.tensor_scalar_mul(A_b[:, cc, :], pa[:, 0:256], float(SCALE))

    # U[c, e] = sum_d Wv[c, d] Wo[d, e]  (fused V+output projection)
    U_b = const.tile([128, 2, 256], BF16, tag="ub")
    for cc in range(2):
        pu = pgv.tile([128, 512], F32, tag="g")
        for dc in range(2):
            nc.tensor.matmul(pu[:, 0:256], WvT[:, dc, cc * 128:(cc + 1) * 128],
                             Wo_b[:, dc, :], start=(dc == 0), stop=(dc == 1))
        nc.any.tensor_copy(U_b[:, cc, :], pu[:, 0:256])

    wv_b = bo2_bc = None
    if use_bias:
        bq_b = const.tile([128, 2], BF16, tag="bqb")
        bv_b = const.tile([128, 2], BF16, tag="bvb")
        for bname, bt in (("bq", bq_b), ("bv", bv_b)):
            bf = wtmp.tile([128, 2], F32, tag="bcol")
            for cc in range(2):
                nc.sync.dma_start(out=bf[:, cc:cc + 1],
                                  in_=b_d[bname][cc * 128:(cc + 1) * 128].unsqueeze(1))
            nc.any.tensor_copy(bt, bf)

        # wv[c] = s * sum_d Wk[c, d] bq[d]
        wv_b = const.tile([128, 2], BF16, tag="wvvec")
        for cc in range(2):
            pwv = psc.tile([128, 4, 128], F32, tag="sc")
            for dc in range(2):
                nc.tensor.matmul(pwv[:, 0, 0:1], WkT[:, dc, cc * 128:(cc + 1) * 128],
                                 bq_b[:, dc:dc + 1], start=(dc == 0), stop=(dc == 1))
            nc.any.tensor_scalar_mul(wv_b[:, cc:cc + 1], pwv[:, 0, 0:1], float(SCALE))

        # bo2[e] = bv @ Wo + bo  (fused output bias)
        bo_f = wtmp.tile([1, 256], F32, tag="borow")
        nc.sync.dma_start(out=bo_f, in_=b_d["bo"][:].unsqueeze(0))
        pb = pgv.tile([128, 512], F32, tag="g")
        for cc in range(2):
            nc.tensor.matmul(pb[0:1, 0:256], bv_b[:, cc:cc + 1], Wo_b[:, cc, :],
                             start=(cc == 0), stop=(cc == 1))
        bo2_f = wtmp.tile([1, 256], F32, tag="bo2row")
        nc.vector.tensor_add(bo2_f, pb[0:1, 0:256], bo_f)
        bo2_dram = nc.dram_tensor("bo2_bounce", [256], F32)
        nc.sync.dma_start(out=bo2_dram[:].unsqueeze(0), in_=bo2_f)
        bo2_bc = const.tile([128, 256], F32, tag="bo2bc")
        bo2_src = bass.AP(tensor=bo2_dram, offset=0,
                          ap=[[0, 128], [1, 256]])
        nc.sync.dma_start(out=bo2_bc, in_=bo2_src)

    # ---------------- main loop ----------------
    # Three phases per chunk: A = load/transpose/G/xU, B = scores+exp,
    # C = attention ladder (attn matmul, 1/rowsum, residual, store).
    # C(ch) is latency-bound (each step waits on the previous engine), so
    # it is interleaved with A(ch+1) at 4-row-tile granularity: the PE
    # always has ready A-work between dependent C-matmuls.
    xbpool = pool("xb", 3)
    xtpool = pool("xt", 2)
    gtpool = pool("gt", 2)
    xupool = pool("xu", 2)
    expool = pool("expt", 9)
    rspool = pool("rs", 6)
    outpool = pool("outs", 3)

    NG = N_RT // 4            # 4-row-tile groups per chunk

    state = {}

    def emit_load(ch):
        r0 = ch * CHUNK
        xb = xbpool.tile([128, N_RT, 256], BF16, tag="xball")
        for g in range(N_RT // 4):
            rr = r0 + g * 512
            nc.gpsimd.dma_start(
                out=xb[:, g * 4:(g + 1) * 4, :],
                in_=x_d[rr:rr + 512, :].rearrange("(t p) c -> p t c", p=128))
        xT = xtpool.tile([128, 2, CHUNK], BF16, tag="xt")
        GT = gtpool.tile([128, 2, CHUNK], BF16, tag="gt")
        xu = xupool.tile([128, N_RT, XU_W], BF16, tag="xub")
        st = {"xb": xb, "r0": r0, "xT": xT, "GT": GT, "xu": xu,
              "expT": [], "xv": None}
        nc.gpsimd.memset(st["xu"][:, :, 256:257], 1.0)
        if use_bias:
            xv_b = xupool.tile([1, CHUNK], BF16, tag="xvb")
            st["xv"] = xv_b
        state[ch] = st
        return st

    def emit_A_unit(ch, g):
        """transposes + G block + xU matmuls for 4 row-tiles."""
        st = state[ch]
        xb, xT, GT, xu = st["xb"], st["xT"], st["GT"], st["xu"]
        rt0 = g * 4
        tpx = ptx.tile([128, 4, 2, 128], BF16, tag="tx")
        for r in range(4):
            for cc in range(2):
                nc.tensor.transpose(tpx[:, r, cc, :],
                                    xb[:, rt0 + r, cc * 128:(cc + 1) * 128],
                                    ident_b)
        dst = xT[:, :, rt0 * 128:(rt0 + 4) * 128].rearrange(
            "p c (r u) -> p r c u", r=4).bitcast(F32)
        if g % 2 == 0:
            nc.scalar.copy(dst, tpx.bitcast(F32))
        else:
            nc.vector.tensor_copy(dst, tpx.bitcast(F32))

        cs = g * 512
        for ac in range(2):
            pgt = pgv.tile([128, 512], F32, tag="g")
            for cc in range(2):
                nc.tensor.matmul(pgt, A_b[:, cc, ac * 128:(ac + 1) * 128],
                                 xT[:, cc, cs:cs + 512],
                                 start=(cc == 0), stop=(cc == 1))
            if ac == 0:
                nc.vector.tensor_copy(GT[:, ac, cs:cs + 512], pgt)
            else:
                nc.scalar.copy(GT[:, ac, cs:cs + 512], pgt)
        if use_bias:
            pxv = pgv.tile([128, 512], F32, tag="g")
            for cc in range(2):
                nc.tensor.matmul(pxv[0:1, :], wv_b[:, cc:cc + 1],
                                 xT[:, cc, cs:cs + 512],
                                 start=(cc == 0), stop=(cc == 1))
            nc.any.tensor_copy(st["xv"][0:1, cs:cs + 512], pxv[0:1, :])

        for r in range(4):
            rt = rt0 + r
            if rt % 2 == 0:
                pxt = pxu.tile([128, 2, 256], F32, tag="xup")
                st["pxt"] = pxt
            ir = rt * 128
            for cc in range(2):
                nc.tensor.matmul(st["pxt"][:, rt % 2, :], xT[:, cc, ir:ir + 128],
                                 U_b[:, cc, :], start=(cc == 0), stop=(cc == 1))
            if rt % 2 == 1:
                dst = xu[:, rt - 1:rt + 1, 0:256]
                if (rt // 2) % 2 == 0:
                    nc.vector.tensor_copy(dst, st["pxt"])
                else:
                    nc.scalar.copy(dst, st["pxt"])

    def emit_B_unit(ch, sg):
        """scores + exp for one 4-row-tile super-group."""
        st = state[ch]
        xT, GT = st["xT"], st["GT"]
        scT4 = psc.tile([128, 4, 128], F32, tag="sc")
        nmm = 12 if use_bias else 8
        mi = 0
        for r in range(4):
            ir = (sg * 4 + r) * 128
            for ac in range(2):
                nc.tensor.matmul(scT4[:, r, :], xT[:, ac, ir:ir + 128],
                                 GT[:, ac, ir:ir + 128],
                                 start=(mi == 0), stop=(mi == nmm - 1))
                mi += 1
            if use_bias:
                nc.tensor.matmul(scT4[:, r, :], st["xv"][0:1, ir:ir + 128],
                                 ones_b, start=False, stop=(mi == nmm - 1))
                mi += 1
        expT4 = expool.tile([128, 4, 128], BF16, tag="expt")
        nc.gpsimd.memset(expT4, 0.0)
        for il in range(2):
            dg = slice(il * 64, (il + 1) * 64)
            nc.scalar.activation(expT4[dg, :, dg], scT4[dg, :, dg],
                                 mybir.ActivationFunctionType.Exp)
        st["expT"].append(expT4)

    def emit_C_unit(ch, sg):
        """attention ladder for one 4-row-tile super-group."""
        st = state[ch]
        for r in range(4):
            rt = sg * 4 + r
            pO = ppo.tile([128, 257], F32, tag="po")
            nc.tensor.matmul(pO, st["expT"][sg][:, r, :],
                             st["xu"][:, rt, 0:257], start=True, stop=True)
            rrs_col = rspool.tile([128, 1], F32, tag="rrs")
            nc.vector.reciprocal(rrs_col, pO[:, 256:257])
            if rt % 2 == 0:
                o_sb2 = outpool.tile([128, 2, 256], F32, tag="osb")
                st["osb"] = o_sb2
            nc.vector.scalar_tensor_tensor(
                st["osb"][:, rt % 2, :], pO[:, 0:256], rrs_col,
                st["xb"][:, rt, :],
                op0=mybir.AluOpType.mult, op1=mybir.AluOpType.add)
            if use_bias:
                nc.gpsimd.tensor_add(st["osb"][:, rt % 2, :],
                                     st["osb"][:, rt % 2, :], bo2_bc)
            if rt % 2 == 1:
                rr = st["r0"] + (rt - 1) * 128
                nc.sync.dma_start(
                    out=out_d[rr:rr + 256, :].rearrange(
                        "(t p) c -> p t c", p=128),
                    in_=st["osb"])

    # prologue: A(0), B(0)
    emit_load(0)
    for g in range(NG):
        emit_A_unit(0, g)
    for sg in range(NG):
        emit_B_unit(0, sg)

    for ch in range(n_chunks):
        nxt = ch + 1 if ch + 1 < n_chunks else None
        if nxt is not None:
            emit_load(nxt)
        for g in range(NG):
            emit_C_unit(ch, g)
            if nxt is not None:
                emit_A_unit(nxt, g)
        if nxt is not None:
            for sg in range(NG):
                emit_B_unit(nxt, sg)
        del state[ch]


def build(n_chunks=RPC // CHUNK, use_bias=True):
    nc = bacc.Bacc("TRN2", target_bir_lowering=False, debug=False)
    rows = n_chunks * CHUNK
    x_d = nc.declare_dram_parameter("x", [rows, C], F32, isOutput=False)
    w_d = {n: nc.declare_dram_parameter(n, [C, C], F32, isOutput=False)
           for n in ("Wq", "Wk", "Wv", "Wo")}
    b_d = {n: nc.declare_dram_parameter(n, [C], F32, isOutput=False)
           for n in ("bq", "bk", "bv", "bo")}
    out_d = nc.declare_dram_parameter("out", [rows, C], F32, isOutput=True)
    from contextlib import ExitStack
    with tile.TileContext(nc) as tc, ExitStack() as ctx:
        _build_body(nc, tc, x_d, w_d, b_d, out_d, n_chunks, ctx, use_bias)
    nc.compile()
    return nc


_NC = {}
TRACE = False
LAST_RESULT = None


def kernel(x, Wq, bq, Wk, bk, Wv, bv, Wo, bo):
    global LAST_RESULT
    use_bias = any(np.any(np.asarray(b)) for b in (bq, bk, bv, bo))
    if use_bias not in _NC:
        _NC[use_bias] = build(use_bias=use_bias)
    nc_k = _NC[use_bias]
    from concourse.bass_utils import run_bass_kernel_spmd

    x = np.ascontiguousarray(np.asarray(x, dtype=np.float32))
    shared = {
        "Wq": np.ascontiguousarray(Wq, dtype=np.float32),
        "Wk": np.ascontiguousarray(Wk, dtype=np.float32),
        "Wv": np.ascontiguousarray(Wv, dtype=np.float32),
        "Wo": np.ascontiguousarray(Wo, dtype=np.float32),
        "bq": np.ascontiguousarray(bq, dtype=np.float32),
        "bk": np.ascontiguousarray(bk, dtype=np.float32),
        "bv": np.ascontiguousarray(bv, dtype=np.float32),
        "bo": np.ascontiguousarray(bo, dtype=np.float32),
    }
    in_maps = []
    for i in range(N_CORES):
        xs = np.ascontiguousarray(
            x[i * BPC:(i + 1) * BPC].reshape(RPC, C))
        in_maps.append({"x": xs, **shared})
    res = run_bass_kernel_spmd(nc_k, in_maps, core_ids=list(range(N_CORES)),
                               trace=TRACE)
    LAST_RESULT = res
    out = np.concatenate(
        [res.results[i]["out"].reshape(BPC, H, W, C) for i in range(N_CORES)],
        axis=0)
    return out
